# revision 1
# baseline (speedup 1.0000x reference)
"""Trainium2 Bass kernel for nn_CDEModel (neural CDE, RK4 over cubic-spline control).

Strategy (pure data parallel over batch, 8 cores x 512 rows):
  * Host precomputes G_u [127,128] matrices mapping knot values x -> spline
    derivative dX(u) per interval for the 5 RK4 sample points u.
  * Phase 1: transpose-load x = [t, a] to T-layout xT [128L, ch*512b] (fp32).
  * Phase 2: dX_u = G_u matmuls (fp32) -> f32r DRAM scratch (action channels
    only; the t channel has dX == 1 exactly and is folded into the k-PSUM
    accumulation directly).
  * Phase 3: encoder z0, stream-stacked T-layout z [128, 256] (two batch
    streams of 256 on partition halves).
  * Scan (127 intervals x 2 substeps x 4 RK stages, 2 batch streams):
      per stream: mm1 (PE, f32r) -> relu+b1 (ACT) -> x0-chunk matmul straight
      into k-PSUM + 4x { pair-chunk mm2 (PE) -> (f+b2)*dX (DVE STT vs
      DMA-broadcast dX tiles) -> selector-matmul accumulating k } ;
      then stacked z updates (DVE scalar_tensor_tensor on [128, 256]).
  * Decoder matmul per interval; output staged T-layout; host transposes.
"""

import sys

import numpy as np

sys.path.insert(0, "/opt/trn_rl_repo")

# ---- problem constants (hardcoded per contract) ----
B = 4096
L = 128
SD = 32          # state dim
AD = 8           # action dim
LD = 64          # latent dim
HID = 128        # hidden dim
XD = AD + 1      # control channels (t + actions)
NCORES = 8
BC = B // NCORES          # 512 batch rows per core
NS = 2                    # batch streams per core
BS = BC // NS             # 256
NI = L - 1                # 127 intervals
NU = 5                    # u grid {0,.25,.5,.75,1}
UVALS = [0.0, 0.25, 0.5, 0.75, 1.0]

_CACHE = {}
LAST_RESULTS = None


def _host_consts():
    n = L - 2
    M = 4.0 * np.eye(n) + np.eye(n, k=1) + np.eye(n, k=-1)
    Minv = np.linalg.inv(M)
    D2 = np.zeros((n, L))
    for i in range(n):
        D2[i, i], D2[i, i + 1], D2[i, i + 2] = 1.0, -2.0, 1.0
    Pfull = np.zeros((L, L))
    Pfull[1:L - 1, :] = 6.0 * (Minv @ D2)
    S0 = np.eye(L)[: L - 1, :]
    S1 = np.eye(L)[1:, :]
    Delta = np.zeros((L - 1, L))
    for i in range(L - 1):
        Delta[i, i], Delta[i, i + 1] = -1.0, 1.0
    Gt = np.zeros((NU, L, NI), np.float32)
    for ui, u in enumerate(UVALS):
        al = -1.0 / 3.0 + u - u * u / 2.0
        be = -1.0 / 6.0 + u * u / 2.0
        G = Delta + al * (S0 @ Pfull) + be * (S1 @ Pfull)
        Gt[ui] = G.T.astype(np.float32)

    ident = np.eye(L, dtype=np.float32)
    I64 = np.eye(64, dtype=np.float32)
    Spair = np.concatenate([I64, I64], axis=0)   # [128, 64]
    return Gt, ident, Spair


def _perm_w2(f_W2, f_b2):
    """Split W2 into the t-channel block (x=0) and 4 action pair blocks.

    W2x0 [128, 64]: cols l -> f_W2[:, l*9].
    W2pr [128, 512]: block pt covers x_lo=1+2pt (cols 0..63 of the block)
    and x_hi=2+2pt (cols 64..127).
    """
    W2x0 = np.ascontiguousarray(f_W2[:, 0::XD])            # [128, 64]
    W2pr = np.empty((HID, 4 * HID), np.float32)
    b2T = np.zeros((128, 4), np.float32)
    for pt in range(4):
        for j in range(128):
            x = (1 + 2 * pt) if j < 64 else (2 + 2 * pt)
            l = j % 64
            W2pr[:, pt * 128 + j] = f_W2[:, l * XD + x]
            b2T[j, pt] = f_b2[l * XD + x]
    b2x0 = np.zeros((1, 128), np.float32)
    b2x0[0, 0:64] = f_b2[0::XD]
    b2x0[0, 64:128] = f_b2[0::XD]
    return W2x0, W2pr, b2T, b2x0


def _pad_encw(enc_W):
    out = np.zeros((65, LD), np.float32)
    out[0:40] = enc_W[0:40]
    out[64] = enc_W[40]
    return out


def _build_program():
    import concourse.bacc as bacc
    import concourse.bass as bass
    import concourse.mybir as mybir
    import concourse.tile as tile
    from contextlib import ExitStack

    dt = mybir.dt
    F32 = dt.float32
    F32R = dt.float32r
    AF = mybir.ActivationFunctionType
    ALU = mybir.AluOpType

    nc = bacc.Bacc("TRN2", target_bir_lowering=False, debug=False,
                   num_devices=NCORES)

    # ---- DRAM tensors ----
    s0_d = nc.dram_tensor("s0", [BC, SD], F32, kind="ExternalInput").ap()
    a_d = nc.dram_tensor("a_in", [BC, L, AD], F32, kind="ExternalInput").ap()
    t_d = nc.dram_tensor("t_in", [BC, L], F32, kind="ExternalInput").ap()
    W1_d = nc.dram_tensor("W1", [LD, HID], F32R, kind="ExternalInput").ap()
    b1_d = nc.dram_tensor("b1", [HID, 1], F32, kind="ExternalInput").ap()
    W2x0_d = nc.dram_tensor("W2x0", [HID, LD], F32R, kind="ExternalInput").ap()
    W2pr_d = nc.dram_tensor("W2pr", [HID, 4 * HID], F32R, kind="ExternalInput").ap()
    b2T_d = nc.dram_tensor("b2T", [128, 4], F32, kind="ExternalInput").ap()
    b2x0_d = nc.dram_tensor("b2x0", [1, 128], F32R, kind="ExternalInput").ap()
    onesr_d = nc.dram_tensor("onesr", [1, BC], F32R, kind="ExternalInput").ap()
    encW_d = nc.dram_tensor("encW", [65, LD], F32R, kind="ExternalInput").ap()
    encb_d = nc.dram_tensor("encb", [LD, 1], F32, kind="ExternalInput").ap()
    decW_d = nc.dram_tensor("decW", [LD, SD], F32R, kind="ExternalInput").ap()
    decb_d = nc.dram_tensor("decb", [SD, 1], F32, kind="ExternalInput").ap()
    Gt_d = nc.dram_tensor("Gt", [NU, L, NI], F32, kind="ExternalInput").ap()
    id_d = nc.dram_tensor("ident", [L, L], F32, kind="ExternalInput").ap()
    Spair_d = nc.dram_tensor("Spair", [128, 64], F32R, kind="ExternalInput").ap()
    zpad_d = nc.dram_tensor("zpad", [24, BC], F32R, kind="ExternalInput").ap()

    outT_d = nc.dram_tensor("outT", [L, SD, BC], F32, kind="ExternalOutput").ap()
    # dX scratch: [interval, 16 padded channel rows, u, batch]; only rows
    # 1..8 (the action channels) are written/used.
    dx_d = nc.dram_tensor("dx_scratch", [NI, 16, NU, BC], F32R).ap()

    mmr = nc.tensor.matmul

    with tile.TileContext(nc, trace_sim=False) as tc, ExitStack() as st:
        # ---------- persistent pools ----------
        wp = st.enter_context(tc.tile_pool(name="weights", bufs=1))

        def wtile(name, dram, shape, dtp):
            t = wp.tile(shape, dtp, tag=name)
            nc.sync.dma_start(t[:], dram)
            return t

        W1_s = wtile("W1", W1_d, [LD, HID], F32R)
        b1_s = wtile("b1", b1_d, [HID, 1], F32)
        W2x0_s = wtile("W2x0", W2x0_d, [HID, LD], F32R)
        W2pr_s = wtile("W2pr", W2pr_d, [HID, 4 * HID], F32R)
        b2T_s = wtile("b2T", b2T_d, [128, 4], F32)
        b2x0_s = wtile("b2x0", b2x0_d, [1, 128], F32R)
        onesr_s = wtile("onesr", onesr_d, [1, BC], F32R)
        encW_s = wtile("encW", encW_d, [65, LD], F32R)
        encb_s = wtile("encb", encb_d, [LD, 1], F32)
        decW_s = wtile("decW", decW_d, [LD, SD], F32R)
        decb_s = wtile("decb", decb_d, [SD, 1], F32)
        id_s = wtile("ident", id_d, [L, L], F32)
        Spair_s = wtile("Spair", Spair_d, [128, 64], F32R)
        Gt_s = wp.tile([L, NU * NI], F32, tag="Gt")
        for u in range(NU):
            nc.sync.dma_start(Gt_s[:, u * NI:(u + 1) * NI], Gt_d[u])

        zp = st.enter_context(tc.tile_pool(name="zstate", bufs=2))
        zst = zp.tile([LD, BC], F32R, tag="z")   # latent, batch-wide T-layout

        # ---------- phases 1-3 ----------
        with tc.tile_pool(name="ph_sb", bufs=4) as php, \
             tc.tile_pool(name="ph_ps", bufs=4, space="PSUM") as ppp:
            xT = php.tile([L, XD * BC], F32, tag="xT")
            in0T = php.tile([65, BC], F32R, tag="in0T")
            nc.sync.dma_start(in0T[40:64, :], zpad_d)
            for cb in range(4):
                csl = slice(cb * 128, (cb + 1) * 128)
                tb = php.tile([128, L], F32, tag="tb")
                nc.sync.dma_start(tb[:], t_d[csl, :])
                ab = php.tile([128, L * AD], F32, tag="ab")
                nc.sync.dma_start(ab[:], a_d[csl].rearrange("b l c -> b (l c)"))
                sb = php.tile([128, SD], F32, tag="sb")
                nc.sync.dma_start(sb[:], s0_d[csl, :])

                a3 = ab[:].rearrange("b (l c) -> b l c", c=AD)
                for ch in range(AD):
                    pa = ppp.tile([L, 128], F32, tag="tp")
                    nc.tensor.transpose(pa[:], a3[:, :, ch], id_s[:])
                    o = (1 + ch) * BC + cb * 128
                    nc.scalar.copy(xT[:, o:o + 128], pa[:])
                ps = ppp.tile([SD, 128], F32, tag="tp")
                nc.tensor.transpose(ps[:], sb[:], id_s[:])
                nc.scalar.copy(in0T[0:SD, cb * 128: cb * 128 + 128], ps[:])
                pa0 = ppp.tile([AD, 128], F32, tag="tp")
                nc.tensor.transpose(pa0[:], a3[:, 0, :], id_s[:])
                nc.scalar.copy(in0T[SD:SD + AD, cb * 128: cb * 128 + 128], pa0[:])
                pt0 = ppp.tile([1, 128], F32, tag="tp")
                nc.tensor.transpose(pt0[:], tb[:, 0:1], id_s[:])
                nc.scalar.copy(in0T[64:65, cb * 128: cb * 128 + 128], pt0[:])

            # phase 2: dX for action channels (full fp32 matmuls)
            for u in range(NU):
                for ch in range(1, XD):
                    pg = ppp.tile([NI, BC], F32, tag="g")
                    mmr(pg[:], Gt_s[:, u * NI:(u + 1) * NI],
                        xT[:, ch * BC:(ch + 1) * BC], start=True, stop=True)
                    gsb = php.tile([NI, BC], F32R, tag="gsb")
                    nc.scalar.copy(gsb[:], pg[:])
                    nc.sync.dma_start(dx_d[:, ch, u, :], gsb[:])

            # phase 3: encoder z0 -> stream-stacked
            pz = ppp.tile([LD, BC], F32, tag="g")
            mmr(pz[:], encW_s[:], in0T[:], start=True, stop=True)
            nc.scalar.activation(zst[:], pz[:], AF.Identity, bias=encb_s[:])
            # decode l=0
            po = ppp.tile([SD, BC], F32, tag="g")
            mmr(po[:], decW_s[:], zst[:], start=True, stop=True)
            oT0 = php.tile([SD, BC], F32, tag="oT")
            nc.scalar.activation(oT0[:], po[:], AF.Identity, bias=decb_s[:])
            nc.sync.dma_start(outT_d[0], oT0[:])

        tc.strict_bb_all_engine_barrier()

        # ---------- scan pools ----------
        ph = st.enter_context(tc.tile_pool(name="ps_h", bufs=2, space="PSUM"))
        pf = st.enter_context(tc.tile_pool(name="ps_f", bufs=3, space="PSUM"))
        pk = st.enter_context(tc.tile_pool(name="ps_k", bufs=2, space="PSUM"))
        pm = st.enter_context(tc.tile_pool(name="ps_m", bufs=1, space="PSUM"))
        hp = st.enter_context(tc.tile_pool(name="h_sb", bufs=3))
        pp = st.enter_context(tc.tile_pool(name="p_sb", bufs=3))
        ztp = st.enter_context(tc.tile_pool(name="zt_sb", bufs=2))
        zap = st.enter_context(tc.tile_pool(name="za_sb", bufs=2))
        bcp = st.enter_context(tc.tile_pool(name="bc_sb", bufs=1))
        fcp = st.enter_context(tc.tile_pool(name="fc_sb", bufs=3))
        otp = st.enter_context(tc.tile_pool(name="o_sb", bufs=2))

        STT = nc.vector.scalar_tensor_tensor
        import os
        NGP = int(os.environ.get("K_NGP", "2"))   # pair-tiles routed to GPSIMD

        import os as _os
        NI_RUN = int(_os.environ.get("K_NI", str(NI)))
        NREP = int(_os.environ.get("K_REP", "1"))
        for _rep in range(NREP):
            zcur = zst
            for i in range(NI_RUN):
                # dX broadcast tiles via replicating DMA: bc[u] [128, 4*BC];
                # block pt at cols [pt*BC,(pt+1)*BC): rows 0:64 <- dx[1+2pt],
                # rows 64:128 <- dx[2+2pt].  One DMA per partition-half: the
                # in-AP walks (rep 64 x pair 4 x batch 512) with a 0-stride
                # replication dim; channel-pair stride is 2 rows = 2*NU*BC.
                bc = []
                XSTRIDE = 2 * NU * BC
                for u in range(NU):
                    bt = bcp.tile([128, 4 * BC], F32R, tag=f"bc{u}")
                    lo = dx_d[i, 1, u, :]
                    hi = dx_d[i, 2, u, :]
                    nc.sync.dma_start(
                        bt[0:64, :],
                        bass.AP(lo.tensor, lo.offset,
                                [[0, 64], [XSTRIDE, 4], [1, BC]]))
                    nc.sync.dma_start(
                        bt[64:128, :],
                        bass.AP(hi.tensor, hi.offset,
                                [[0, 64], [XSTRIDE, 4], [1, BC]]))
                    bc.append(bt)

                for sub in range(2):
                    uix = [0, 1, 1, 2] if sub == 0 else [2, 3, 3, 4]
                    znew = zp.tile([LD, BC], F32R, tag="z")
                    ztmp = None
                    zacc = None
                    for s in range(4):
                        zin = zcur if s == 0 else ztmp
                        # hidden layer, both streams into one wide PSUM tile
                        hps = ph.tile([HID, BC], F32, tag="h")
                        mmr(hps[:, 0:BS], W1_s[:], zin[:, 0:BS], start=True,
                            stop=True)
                        mmr(hps[:, BS:BC], W1_s[:], zin[:, BS:BC], start=True,
                            stop=True)
                        hsb = hp.tile([HID, BC], F32R, tag="h")
                        nc.scalar.activation(hsb[:], hps[:], AF.Relu, bias=b1_s[:])
                        # k accumulation: b2(t-ch) seed + t-channel f + pair sums
                        kps = pk.tile([LD, BC], F32, tag="k")
                        mmr(kps[:], b2x0_s[:, 0:LD], onesr_s[:], start=True,
                            stop=False, skip_group_check=True)
                        mmr(kps[:, 0:BS], W2x0_s[:], hsb[:, 0:BS], start=False,
                            stop=False, skip_group_check=True)
                        mmr(kps[:, BS:BC], W2x0_s[:], hsb[:, BS:BC], start=False,
                            stop=False, skip_group_check=True)
                        for pt in range(4):
                            fps = pf.tile([HID, BC], F32, tag="f")
                            mmr(fps[:], W2pr_s[:, pt * 128:(pt + 1) * 128],
                                hsb[:], start=True, stop=True)
                            psb = pp.tile([HID, BC], F32R, tag="p")
                            bslc = bc[uix[s]][:, pt * BC:(pt + 1) * BC]
                            if pt < NGP:
                                fsb = fcp.tile([HID, BC], F32, tag="fc")
                                nc.scalar.activation(fsb[:], fps[:], AF.Identity,
                                                     bias=b2T_s[:, pt:pt + 1])
                                nc.gpsimd.tensor_tensor(
                                    psb[:], fsb[:], bslc, op=ALU.mult)
                            else:
                                STT(psb[:], fps[:], b2T_s[:, pt:pt + 1], bslc,
                                    op0=ALU.add, op1=ALU.mult)
                            mmr(kps[:, 0:BS], Spair_s[:], psb[:, 0:BS],
                                start=False, stop=False, skip_group_check=True)
                            mmr(kps[:, BS:BC], Spair_s[:], psb[:, BS:BC],
                                start=False, stop=(pt == 3),
                                skip_group_check=True)
                        # wide updates
                        if s == 0:
                            zacc = zap.tile([LD, BC], F32, tag="za")
                            nc.scalar.copy(zacc[:], kps[:])
                            ztmp = ztp.tile([LD, BC], F32R, tag="zt")
                            STT(ztmp[:], kps[:], 0.25, zcur[:],
                                op0=ALU.mult, op1=ALU.add)
                        elif s in (1, 2):
                            za2 = zap.tile([LD, BC], F32, tag="za")
                            STT(za2[:], kps[:], 2.0, zacc[:],
                                op0=ALU.mult, op1=ALU.add)
                            zacc = za2
                            ztmp2 = ztp.tile([LD, BC], F32R, tag="zt")
                            STT(ztmp2[:], kps[:], 0.25 if s == 1 else 0.5,
                                zcur[:], op0=ALU.mult, op1=ALU.add)
                            ztmp = ztmp2
                        else:
                            za2 = zap.tile([LD, BC], F32, tag="za")
                            STT(za2[:], kps[:], 1.0, zacc[:],
                                op0=ALU.mult, op1=ALU.add)
                            STT(znew[:], za2[:], 1.0 / 12.0, zcur[:],
                                op0=ALU.mult, op1=ALU.add)
                    zcur = znew

                # decode z_{i+1}
                pdo = pm.tile([SD, BC], F32, tag="m")
                mmr(pdo[:], decW_s[:], zcur[:], start=True, stop=True)
                oT = otp.tile([SD, BC], F32, tag="oT")
                nc.scalar.activation(oT[:], pdo[:], AF.Identity, bias=decb_s[:])
                nc.sync.dma_start(outT_d[i + 1], oT[:])

    nc.compile()
    return nc


def _get_program():
    if "nc" not in _CACHE:
        _CACHE["nc"] = _build_program()
    return _CACHE["nc"]


def build_in_maps(s, a, t, enc_W, enc_b, f_W1, f_b1, f_W2, f_b2, dec_W, dec_b):
    s = np.ascontiguousarray(np.asarray(s, np.float32))
    a = np.ascontiguousarray(np.asarray(a, np.float32))
    t = np.ascontiguousarray(np.asarray(t, np.float32))
    Gt, ident, Spair = _host_consts()
    W2x0, W2pr, b2T, b2x0 = _perm_w2(np.asarray(f_W2, np.float32),
                                     np.asarray(f_b2, np.float32))
    const_map = dict(
        W1=np.ascontiguousarray(np.asarray(f_W1, np.float32)),
        b1=np.asarray(f_b1, np.float32).reshape(HID, 1).copy(),
        W2x0=W2x0, W2pr=W2pr, b2T=b2T, b2x0=b2x0,
        onesr=np.ones((1, BC), np.float32),
        encW=_pad_encw(np.asarray(enc_W, np.float32)),
        encb=np.asarray(enc_b, np.float32).reshape(LD, 1).copy(),
        decW=np.ascontiguousarray(np.asarray(dec_W, np.float32)),
        decb=np.asarray(dec_b, np.float32).reshape(SD, 1).copy(),
        Gt=Gt, ident=ident, Spair=Spair,
        zpad=np.zeros((24, BC), np.float32),
    )
    in_maps = []
    for c in range(NCORES):
        rs = slice(c * BC, (c + 1) * BC)
        m = dict(const_map)
        m["s0"] = np.ascontiguousarray(s[rs, 0, :])
        m["a_in"] = np.ascontiguousarray(a[rs])
        m["t_in"] = np.ascontiguousarray(t[rs])
        in_maps.append(m)
    return in_maps


def kernel(s, a, t, enc_W, enc_b, f_W1, f_b1, f_W2, f_b2, dec_W, dec_b):
    global LAST_RESULTS
    from concourse.bass_utils import run_bass_kernel_spmd

    in_maps = build_in_maps(s, a, t, enc_W, enc_b, f_W1, f_b1, f_W2, f_b2,
                            dec_W, dec_b)
    nc = _get_program()
    res = run_bass_kernel_spmd(nc, in_maps, core_ids=list(range(NCORES)))
    LAST_RESULTS = res

    out = np.empty((B, L, SD), np.float32)
    for c in range(NCORES):
        oT = res.results[c]["outT"]          # [L, SD, BC]
        out[c * BC:(c + 1) * BC] = oT.transpose(2, 0, 1)
    return out

